# revision 1
# baseline (speedup 1.0000x reference)
"""Causal self-attention (GQA, RoPE) on 8 Trainium2 NeuronCores.

Sharding: tensor-parallel by KV-head group. Core c owns kv-head c and its 4
query heads, for both batch elements. Each core computes:
  qkv^T slice -> RoPE -> causal attention -> out-projection partial
The host sums the 8 partial out-projection results (Wout row-sharded), which
replaces the all-reduce.

All matmuls run in float32r (fp32 with mantissa RNE-rounded to 11 bits; runs
at bf16 speed on the PE). Inputs are pre-rounded on the host with the exact
hardware rounding, so device-side rounding passes are only needed for tensors
the hardware requires a compute-engine producer for (weights, via GPSIMD).

Layouts (per core, s = b*S + pos, SQ = B*S):
  xT    [H, SQ]  f32r   x transposed, host pre-rounded
  w3    [H, 768] f32r   [Wq(4 heads, pre-scaled by 1/sqrt(hd)) | Wk | Wv]
  wout  [512, H] f32r   Wout rows for this core's 4 q heads
  cosT  [128, SQ] f32   cos table transposed, tiled over batches
  sinS  [128, SQ] f32   sin table, rows 0:64 negated (rotate_half baked in)
  maskT [128, 4*512] f32  causal 0/1 mask for the 4 diagonal k-chunks
Scratch (DRAM): qkvT_sp [128, 6, SQ] f32r, attnT_sp [128, 4, SQ] f32r.
Output: outT [H, SQ] fp16 (partial out-projection, transposed; host sums in fp32).
"""
import numpy as np

import concourse.bass as bass
import concourse.mybir as mybir
import concourse.tile as tile
from concourse import bacc
from concourse.masks import make_identity

F32 = mybir.dt.float32
R = mybir.dt.float32r
P = 128

N_CORES = 8
CFG = dict(B=2, S=2048, H=4096, HD=128, NQ=4)  # NQ = q heads per core


def round_f32r(x: np.ndarray) -> np.ndarray:
    """Exact replica of the TRN2 f32r rounding: RNE of the low 12 mantissa bits."""
    x = np.ascontiguousarray(x, dtype=np.float32)
    b = x.view(np.uint32).astype(np.uint64)
    half = np.uint64(1 << 11)
    mask = np.uint64((1 << 12) - 1)
    low = b & mask
    t = b & ~mask
    up = (low > half) | (((low == half) & ((t >> np.uint64(12)) & np.uint64(1))) == 1)
    out = np.where(up, t + np.uint64(1 << 12), t).astype(np.uint32)
    return out.view(np.float32).reshape(x.shape)


def build(cfg=CFG, reps=1, phases=(1, 2, 3)):
    B, S, H, HD, NQ = cfg["B"], cfg["S"], cfg["H"], cfg["HD"], cfg["NQ"]
    SQ = B * S
    HCH = H // P          # h chunks
    C6 = NQ + 2           # c-tiles: NQ q heads, 1 k, 1 v
    CW = C6 * P           # qkv out width per core
    NSB = SQ // 512       # 512-wide s blocks
    NHB = SQ // 256       # 256-wide s half-blocks (phase 1)
    QB = S // 512         # q blocks per batch
    SCH = S // P          # s chunks per batch (k chunks)

    nc = bacc.Bacc("TRN2", target_bir_lowering=False, debug=False,
                   num_devices=N_CORES)
    xT = nc.dram_tensor("xT", [H, SQ], R, kind="ExternalInput").ap()
    w3 = nc.dram_tensor("w3", [H, CW], R, kind="ExternalInput").ap()
    wout = nc.dram_tensor("wout", [NQ * P, H], R, kind="ExternalInput").ap()
    cosT = nc.dram_tensor("cosT", [P, SQ], F32, kind="ExternalInput").ap()
    sinS = nc.dram_tensor("sinS", [P, SQ], F32, kind="ExternalInput").ap()
    maskT = nc.dram_tensor("maskT", [P, 4 * 512], F32, kind="ExternalInput").ap()
    outT = nc.dram_tensor("outT", [H, SQ], mybir.dt.float16, kind="ExternalOutput").ap()

    xT_v = xT.rearrange("(ho p) s -> p ho s", p=P)      # [128, HCH, SQ]
    w3_v = w3.rearrange("(ho p) c -> p ho c", p=P)      # [128, HCH, CW]
    wout_v = wout.rearrange("(co p) n -> p co n", p=P)  # [128, NQ, H]

    with tile.TileContext(nc, pool_alloc_mode="queue") as tc:
        with tc.tile_pool(name="dram", bufs=1, space="DRAM") as dram:
            # per-s-block scratch so later phases can start before earlier
            # phases fully drain (whole-tile DRAM deps otherwise serialize)
            qkvT_sp = [dram.tile([P, C6, 512], R, name=f"qkv_sp{i}")
                       for i in range(NSB)]
            attnT_sp = [dram.tile([P, NQ, 512], R, name=f"att_sp{i}")
                        for i in range(NSB)]

            def body(iv=None):
                # ---------------- Phase 1: qkv^T = w3^T @ x^T ----------------
                if 1 not in phases:
                    pass
                else:
                  with tc.tile_pool(name="p1w", bufs=1) as p1w, \
                     tc.tile_pool(name="p1ws", bufs=2) as p1ws, \
                     tc.tile_pool(name="p1x", bufs=2) as p1x, \
                     tc.tile_pool(name="p1st", bufs=3) as p1st, \
                     tc.tile_pool(name="ps1", bufs=6, space="PSUM") as ps1:
                    w3_r = p1w.tile([P, HCH, CW], R)
                    for g in range(HCH // 2):
                        wst = p1ws.tile([P, 2, CW], F32, name="wst", tag="wst")
                        nc.sync.dma_start(wst[:], w3_v[:, 2 * g:2 * g + 2, :].bitcast(F32))
                        nc.gpsimd.tensor_copy(w3_r[:, 2 * g:2 * g + 2, :], wst[:])

                    for hb in range(NHB):
                        xt = p1x.tile([P, HCH, 256], R, name="xt", tag="xt")
                        nc.sync.dma_start(xt[:], xT_v[:, :, hb * 256:hb * 256 + 256])
                        stage = p1st.tile([P, C6, 256], R, name="stage", tag="stage")
                        for ci in range(C6):
                            ps = ps1.tile([P, 256], F32, name="p1p", tag="p1p")
                            for hc in range(HCH):
                                nc.tensor.matmul(
                                    ps[:],
                                    w3_r[:, hc, ci * P:(ci + 1) * P],
                                    xt[:, hc, :],
                                    start=(hc == 0), stop=(hc == HCH - 1),
                                )
                            nc.vector.tensor_copy(stage[:, ci, :], ps[:])
                        nc.sync.dma_start(
                            qkvT_sp[hb // 2][:, :, (hb % 2) * 256:(hb % 2) * 256 + 256],
                            stage[:])

                # ---------------- Phase 2: attention ----------------
                if 2 not in phases:
                    pass
                else:
                  with tc.tile_pool(name="p2c", bufs=1) as p2c, \
                     tc.tile_pool(name="p2w", bufs=2) as p2w, \
                     tc.tile_pool(name="p2r", bufs=1) as p2r, \
                     tc.tile_pool(name="p2pt", bufs=2) as p2pt, \
                     tc.tile_pool(name="ps2", bufs=1, space="PSUM") as ps2:
                    mask_t = p2c.tile([P, 4, 512], F32)
                    nc.sync.dma_start(mask_t[:], maskT.rearrange("p (v q) -> p v q", v=4))
                    ones_f = p2c.tile([P, P], F32)
                    nc.vector.memset(ones_f[:], 1.0)
                    ones_r = p2c.tile([P, P], R)
                    nc.vector.tensor_copy(ones_r[:], ones_f[:])
                    ident_f = p2c.tile([P, P], F32)
                    make_identity(nc, ident_f[:])
                    ident_r = p2c.tile([P, P], R)
                    nc.vector.tensor_copy(ident_r[:], ident_f[:])

                    kT_r = p2c.tile([P, SQ], R)
                    v_r = p2c.tile([P, B * SCH, HD], R)

                    def rope_block(ci_lo, n, off):
                        """Returns an [P, n, 512] f32r tile holding
                        rope(qkvT_sp[sb][:, ci_lo:ci_lo+n, :]) — one reload DMA,
                        DVE partition-rotated copy, cos/sin broadcast over heads.
                        """
                        h2 = HD // 2
                        sp = qkvT_sp[off // 512]
                        qld = p2r.tile([P, NQ, 512], R, name="rope_ld", tag="rope_ld")
                        nc.sync.dma_start(qld[:, :n, :], sp[:, ci_lo:ci_lo + n, :])
                        qrt = p2r.tile([P, NQ, 512], R, name="rope_rt", tag="rope_rt")
                        nc.vector.tensor_copy(qrt[:h2, :n, :], qld[h2:2 * h2, :n, :])
                        nc.vector.tensor_copy(qrt[h2:2 * h2, :n, :], qld[:h2, :n, :])
                        cs = p2w.tile([P, 512], F32, name="cs", tag="cs")
                        sn = p2w.tile([P, 512], F32, name="sn", tag="sn")
                        nc.sync.dma_start(cs[:], cosT[:, off:off + 512])
                        nc.sync.dma_start(sn[:], sinS[:, off:off + 512])
                        cs_b = cs[:, None, :].to_broadcast((P, n, 512))
                        sn_b = sn[:, None, :].to_broadcast((P, n, 512))
                        t1 = p2r.tile([P, NQ, 512], F32, name="rope_t1", tag="rope_t1")
                        t2 = p2r.tile([P, NQ, 512], F32, name="rope_t2", tag="rope_t2")
                        nc.vector.tensor_mul(t1[:, :n, :], qld[:, :n, :].bitcast(F32), cs_b)
                        nc.vector.tensor_mul(t2[:, :n, :], qrt[:, :n, :].bitcast(F32), sn_b)
                        qr = p2w.tile([P, NQ, 512], R, name="qr", tag="qr")
                        nc.vector.tensor_add(qr[:, :n, :], t1[:, :n, :], t2[:, :n, :])
                        return qr

                    # K rope + V transpose, per batch
                    for b in range(B):
                        for j in range(S // 512):
                            off = b * S + j * 512
                            kr = rope_block(NQ, 1, off)
                            nc.vector.tensor_copy(kT_r[:, off:off + 512], kr[:, 0, :])
                            vsl = p2w.tile([P, 512], R, name="vsl", tag="vsl")
                            nc.sync.dma_start(vsl[:], qkvT_sp[off // 512][:, NQ + 1, :])
                            for jj in range(512 // P):
                                so = (off // P) + jj
                                tps = ps2.tile([P, P], R, name="vt", tag="vt")
                                nc.tensor.transpose(
                                    tps[:], vsl[:, jj * P:(jj + 1) * P], ident_r[:])
                                nc.vector.tensor_copy(v_r[:, so, :], tps[:])

                    for b in range(B):
                        for qb in range(QB):
                            nch = (qb + 1) * 4
                            qoff = b * S + qb * 512
                            qr4 = rope_block(0, NQ, qoff)
                            for h in range(NQ):
                                qr = qr4[:, h, :]
                                pT = p2pt.tile([P, 4 * QB, 512], R, name="pT", tag="pT")
                                # softmax denominator accumulates on the PE via
                                # per-chunk ones-matmuls, issued one chunk behind
                                # the scores so the exp has time to land — keeps
                                # the in-order PE queue from stalling on the
                                # ACT/GPSIMD round-trip.
                                lps = ps2.tile([P, 512], F32, name="lp", tag="lp",
                                               bufs=2)
                                ops = ps2.tile([P, 512], F32, name="av", tag="av",
                                               bufs=2)
                                # issue order maximizes PE-queue distance from the
                                # ACT(exp)/GPSIMD(mask) producers: all scores first,
                                # then denominator+AV matmuls on unmasked chunks,
                                # masked (diagonal) chunks last.
                                for kc in range(nch):
                                    sps = ps2.tile([P, 512], F32, name="sc", tag="sc",
                                                   bufs=3)
                                    nc.tensor.matmul(
                                        sps[:], kT_r[:, b * S + kc * P: b * S + (kc + 1) * P],
                                        qr[:], start=True, stop=True)
                                    nc.scalar.activation(
                                        pT[:, kc, :], sps[:],
                                        mybir.ActivationFunctionType.Exp)
                                    if kc >= nch - 4:
                                        nc.gpsimd.tensor_mul(
                                            pT[:, kc, :],
                                            pT[:, kc, :].bitcast(F32),
                                            mask_t[:, kc - (nch - 4), :])
                                order = list(range(nch - 4)) + list(range(nch - 4, nch))
                                for i, kc in enumerate(order):
                                    nc.tensor.matmul(
                                        lps[:], ones_r[:], pT[:, kc, :],
                                        start=(i == 0), stop=(i == nch - 1))
                                    nc.tensor.matmul(
                                        ops[:], v_r[:, b * SCH + kc, :], pT[:, kc, :],
                                        start=(i == 0), stop=(i == nch - 1))
                                rec = p2w.tile([P, 512], F32, name="rec", tag="rec")
                                nc.vector.reciprocal(rec[:], lps[:])
                                att = p2w.tile([P, 512], R, name="att", tag="att")
                                nc.vector.tensor_mul(att[:], ops[:], rec[:])
                                nc.sync.dma_start(
                                    attnT_sp[qoff // 512][:, h, :], att[:])

                # ---------------- Phase 3: out projection ----------------
                if 3 not in phases:
                    pass
                else:
                  with tc.tile_pool(name="p3w", bufs=1) as p3w, \
                     tc.tile_pool(name="p3ws", bufs=2) as p3ws, \
                     tc.tile_pool(name="p3a", bufs=2) as p3a, \
                     tc.tile_pool(name="p3o", bufs=4) as p3o, \
                     tc.tile_pool(name="ps3", bufs=4, space="PSUM") as ps3:
                    wout_r = p3w.tile([P, NQ, H], R)
                    for g in range(H // 512):
                        wst3 = p3ws.tile([P, NQ, 512], F32, name="wst3", tag="wst3")
                        nc.sync.dma_start(
                            wst3[:], wout_v[:, :, g * 512:(g + 1) * 512].bitcast(F32))
                        nc.gpsimd.tensor_copy(wout_r[:, :, g * 512:(g + 1) * 512], wst3[:])

                    for sb in range(NSB):
                        att_in = p3a.tile([P, NQ, 512], R, name="att_in", tag="att_in")
                        nc.sync.dma_start(att_in[:], attnT_sp[sb][:])
                        for ht in range(H // P):
                            ops3 = ps3.tile([P, 512], F32, name="o3", tag="o3")
                            for ci in range(NQ):
                                nc.tensor.matmul(
                                    ops3[:], wout_r[:, ci, ht * P:(ht + 1) * P],
                                    att_in[:, ci, :],
                                    start=(ci == 0), stop=(ci == NQ - 1))
                            ost = p3o.tile([P, 512], mybir.dt.float16, name="ost", tag="ost")
                            nc.vector.tensor_copy(ost[:], ops3[:])
                            nc.sync.dma_start(
                                outT[ht * P:(ht + 1) * P, sb * 512:(sb + 1) * 512], ost[:])

            if reps == 1:
                body()
            else:
                with tc.For_i(0, reps, 1) as iv:
                    body(iv)
    return nc


def host_inputs(x, cos, sin, Wqkv, Wout, cfg=CFG):
    """Build the 8 per-core input maps from the full-problem inputs."""
    B, S, H, HD, NQ = cfg["B"], cfg["S"], cfg["H"], cfg["HD"], cfg["NQ"]
    SQ = B * S
    NH = NQ * N_CORES          # total q heads
    scale = 1.0 / np.sqrt(HD)

    x = np.asarray(x, dtype=np.float32)
    cos = np.asarray(cos, dtype=np.float32)
    sin = np.asarray(sin, dtype=np.float32)
    Wqkv = np.asarray(Wqkv, dtype=np.float32)
    Wout = np.asarray(Wout, dtype=np.float32)

    xT_r = round_f32r(x.reshape(SQ, H).T)
    cosT2 = np.ascontiguousarray(np.tile(cos.T, (1, B)))
    sinT = sin.T
    sinS2 = np.concatenate([-sinT[:HD // 2], sinT[HD // 2:]], axis=0)
    sinS2 = np.ascontiguousarray(np.tile(sinS2, (1, B)))
    qv = np.arange(512)
    pv = np.arange(P)
    mask = np.zeros((P, 4, 512), np.float32)
    for v in range(4):
        mask[:, v, :] = (qv[None, :] >= (v * P + pv)[:, None])
    mask = mask.reshape(P, 4 * 512)

    in_maps = []
    for c in range(N_CORES):
        wq = Wqkv[:, c * NQ * HD:(c + 1) * NQ * HD] * scale
        wk = Wqkv[:, NH * HD + c * HD: NH * HD + (c + 1) * HD]
        wv = Wqkv[:, NH * HD + N_CORES * HD + c * HD: NH * HD + N_CORES * HD + (c + 1) * HD]
        w3 = round_f32r(np.concatenate([wq, wk, wv], axis=1))
        wout = round_f32r(Wout[c * NQ * HD:(c + 1) * NQ * HD, :])
        in_maps.append({
            "xT": xT_r, "w3": w3, "wout": wout,
            "cosT": cosT2, "sinS": sinS2, "maskT": mask,
        })
    return in_maps


class _Runner:
    """Compiled-kernel runner over the axon PJRT path (kept for re-invocation)."""

    def __init__(self, nc, n_cores):
        import jax
        from jax.sharding import Mesh, PartitionSpec
        from jax.experimental.shard_map import shard_map
        from concourse.bass2jax import (
            _bass_exec_p, partition_id_tensor, install_neuronx_cc_hook)
        install_neuronx_cc_hook()
        self.jax = jax
        self.n_cores = n_cores
        partition_name = nc.partition_id_tensor.name if nc.partition_id_tensor else None
        in_names, out_names, out_avals, zero_outs = [], [], [], []
        for alloc in nc.m.functions[0].allocations:
            if not isinstance(alloc, mybir.MemoryLocationSet):
                continue
            name = alloc.memorylocations[0].name
            if alloc.kind == "ExternalInput":
                if name != partition_name:
                    in_names.append(name)
            elif alloc.kind == "ExternalOutput":
                shape = tuple(alloc.tensor_shape)
                dtype = mybir.dt.np(alloc.dtype)
                out_avals.append(jax.core.ShapedArray(shape, dtype))
                out_names.append(name)
                zero_outs.append(np.zeros(shape, dtype))
        self.in_names = in_names[:]
        self.out_names, self.out_avals, self.zero_outs = out_names, out_avals, zero_outs
        self.n_params = len(in_names)
        all_names = in_names + out_names
        if partition_name is not None:
            all_names.append(partition_name)

        def _body(*args):
            operands = list(args)
            if partition_name is not None:
                operands.append(partition_id_tensor())
            outs = _bass_exec_p.bind(
                *operands, out_avals=tuple(out_avals), in_names=tuple(all_names),
                out_names=tuple(out_names), lowering_input_output_aliases=(),
                sim_require_finite=True, sim_require_nnan=True, nc=nc)
            return tuple(outs)

        devices = jax.devices()[:n_cores]
        self.mesh = Mesh(np.asarray(devices), ("core",))
        specs_in = (PartitionSpec("core"),) * (self.n_params + len(out_names))
        specs_out = (PartitionSpec("core"),) * len(out_names)
        self.sharded = jax.jit(
            shard_map(_body, mesh=self.mesh, in_specs=specs_in,
                      out_specs=specs_out, check_rep=False),
            keep_unused=True)
        self._dev_args = None

    def stage(self, in_maps):
        import jax
        from jax.sharding import PartitionSpec
        per_core = [[np.asarray(m[n]) for n in self.in_names] for m in in_maps]
        concat = [np.concatenate([per_core[c][i] for c in range(self.n_cores)], axis=0)
                  for i in range(self.n_params)]
        concat += [np.zeros((self.n_cores * z.shape[0], *z.shape[1:]), z.dtype)
                   for z in self.zero_outs]
        sh = jax.sharding.NamedSharding(self.mesh, PartitionSpec("core"))
        self._dev_args = [jax.device_put(a, sh) for a in concat]
        jax.block_until_ready(self._dev_args)

    def execute(self):
        out = self.sharded(*self._dev_args)
        self.jax.block_until_ready(out)
        return out

    def results(self, out):
        return [
            {n: np.asarray(out[i]).reshape(self.n_cores, *self.out_avals[i].shape)[c]
             for i, n in enumerate(self.out_names)}
            for c in range(self.n_cores)
        ]


_cached = {}


def _get_runner(reps=1):
    key = reps
    if key not in _cached:
        nc = build(CFG, reps=reps)
        nc.compile()
        _cached[key] = _Runner(nc, N_CORES)
    return _cached[key]


def kernel(x, cos, sin, Wqkv, Wout):
    cfg = CFG
    B, S, H = cfg["B"], cfg["S"], cfg["H"]
    runner = _get_runner(reps=1)
    in_maps = host_inputs(x, cos, sin, Wqkv, Wout, cfg)
    runner.stage(in_maps)
    out = runner.execute()
    results = runner.results(out)
    acc = np.zeros((B * S, H), np.float32)
    for c in range(N_CORES):
        acc += results[c]["outT"].T.astype(np.float32)
    return acc.reshape(B, S, H).astype(np.float32)



# revision 16
# speedup vs baseline: 1.0358x; 1.0358x over previous
"""Causal self-attention (GQA, RoPE) on 8 Trainium2 NeuronCores.

Sharding: tensor-parallel by KV-head group. Core c owns kv-head c and its 4
query heads, for both batch elements. Each core computes:
  qkv^T slice -> RoPE -> causal attention -> out-projection partial
The host sums the 8 partial out-projection results (Wout row-sharded), which
replaces the all-reduce.

All matmuls run in bf16 (PE peak rate, fast weight load); PSUM accumulation is
fp32. Causal masking is an additive -60 bias folded into the scores PSUM via an
identity-weight matmul (exp(-60)~1e-26, vanishes in bf16). The softmax
reciprocal runs on the Scalar engine as exp(-ln(x)) (same ACT table set as the
exp). qkv stays resident in SBUF (no DRAM round-trip); attention output makes
one bf16 round-trip so the out-projection can rerun with 4-block PSUM
accumulators (weight loads amortized over 4 matmuls).

Layouts (per core, s = b*S + pos, SQ = B*S):
  xT    [H, SQ]   bf16  x transposed
  w3    [H, 768]  bf16  [Wq(4 heads, pre-scaled by 1/sqrt(hd)) | Wk | Wv]
  wout  [512, H]  bf16  Wout rows for this core's 4 q heads
  cosT  [128, S]  bf16  cos table transposed (per-position, shared by batches)
  sinS  [128, S]  bf16  sin table, rows 0:64 negated (rotate_half baked in)
  biasT [128, 4*512] bf16  additive causal bias (0 valid / -60 masked) for the
                           4 diagonal k-chunks of a 512-wide q block
Scratch (DRAM): attnT_sp [128, 4, SQ] bf16.
Output: outT [H, SQ] fp16 (partial out-projection, transposed; host sums fp32).
"""
import numpy as np

import concourse.bass as bass
import concourse.mybir as mybir
import concourse.tile as tile
from concourse import bacc
from concourse.masks import make_identity

F32 = mybir.dt.float32
BF = mybir.dt.bfloat16
F16 = mybir.dt.float16
P = 128

N_CORES = 8
CFG = dict(B=2, S=2048, H=4096, HD=128, NQ=4)  # NQ = q heads per core


def build(cfg=CFG, reps=1):
    B, S, H, HD, NQ = cfg["B"], cfg["S"], cfg["H"], cfg["HD"], cfg["NQ"]
    SQ = B * S
    HCH = H // P          # h chunks (contraction tiles)
    C6 = NQ + 2           # c-tiles: NQ q heads, 1 k, 1 v
    CW = C6 * P           # qkv out width per core
    NSB = SQ // 512       # 512-wide s blocks
    QB = S // 512         # q blocks per batch
    SCH = S // P          # k chunks per batch
    h2 = HD // 2

    nc = bacc.Bacc("TRN2", target_bir_lowering=False, debug=False,
                   num_devices=N_CORES)
    xT = nc.dram_tensor("xT", [H, SQ], BF, kind="ExternalInput").ap()
    w3 = nc.dram_tensor("w3", [H, CW], BF, kind="ExternalInput").ap()
    wout = nc.dram_tensor("wout", [NQ * P, H], BF, kind="ExternalInput").ap()
    cosT = nc.dram_tensor("cosT", [P, S], BF, kind="ExternalInput").ap()
    sinS = nc.dram_tensor("sinS", [P, S], BF, kind="ExternalInput").ap()
    biasT = nc.dram_tensor("biasT", [P, 4 * 512], BF, kind="ExternalInput").ap()
    outT = nc.dram_tensor("outT", [H, SQ], F16, kind="ExternalOutput").ap()

    xT_v = xT.rearrange("(ho p) s -> p ho s", p=P)        # [128, HCH, SQ]
    w3_v = w3.rearrange("(ho p) c -> p ho c", p=P)        # [128, HCH, CW]
    wout_v = wout.rearrange("(co p) n -> p co n", p=P)    # [128, NQ, H]
    outT_v = outT.rearrange("(ho p) (hf q) -> p ho hf q", p=P, q=512)

    with tile.TileContext(nc, pool_alloc_mode="queue") as tc:
        with tc.tile_pool(name="dram", bufs=1, space="DRAM") as dram, \
             tc.tile_pool(name="const", bufs=1) as cp:
            attnT_sp = dram.tile([P, NQ, SQ], BF, name="att_sp")

            # ---- persistent constants/weights (outside the reps loop) ----
            w3t = cp.tile([P, HCH, CW], BF)
            nc.sync.dma_start(w3t[:], w3_v)
            cos_t = cp.tile([P, S], BF)
            nc.sync.dma_start(cos_t[:], cosT)
            sin_t = cp.tile([P, S], BF)
            nc.sync.dma_start(sin_t[:], sinS)
            bias_t = cp.tile([P, 4, 512], BF)
            nc.sync.dma_start(bias_t[:], biasT.rearrange("p (v q) -> p v q", v=4))
            ident_b = cp.tile([P, P], BF)
            ones_b = cp.tile([P, P], BF)
            with tc.tile_pool(name="init", bufs=1) as ip:
                idf = ip.tile([P, P], F32)
                make_identity(nc, idf[:])
                nc.vector.tensor_copy(ident_b[:], idf[:])
                onef = ip.tile([P, P], F32)
                nc.vector.memset(onef[:], 1.0)
                nc.vector.tensor_copy(ones_b[:], onef[:])

            def body(iv=None):
              pspool = {}

              def ps_tile(*a, **kw):
                  return pspool["cur"].tile(*a, **kw)

              with tc.tile_pool(name="span", bufs=1) as sp, \
                   tc.tile_pool(name="rp", bufs=1) as rp:
                qkv_sb = sp.tile([P, C6, SQ], BF)      # q|k|v, SBUF-resident
                kT_r = sp.tile([P, SQ], BF)            # roped K
                v_r = sp.tile([P, B * SCH, HD], BF)    # V transposed

                def rope(dst, src_ap, coff, n):
                    """dst[:, :n, 512] = rope(src) using cos/sin cols coff."""
                    qrt = rp.tile([P, n, 512], BF, name="qrt", tag=f"qrt{n}")
                    nc.vector.tensor_copy(qrt[:h2, :, :], src_ap[h2:2 * h2, :n, :])
                    nc.vector.tensor_copy(qrt[h2:2 * h2, :, :], src_ap[:h2, :n, :])
                    cs_b = cos_t[:, coff:coff + 512][:, None, :].to_broadcast((P, n, 512))
                    sn_b = sin_t[:, coff:coff + 512][:, None, :].to_broadcast((P, n, 512))
                    t1 = rp.tile([P, n, 512], BF, name="rt1", tag=f"rt1{n}")
                    t2 = rp.tile([P, n, 512], BF, name="rt2", tag=f"rt2{n}")
                    nc.vector.tensor_mul(t1[:], src_ap[:, :n, :], cs_b)
                    nc.vector.tensor_mul(t2[:], qrt[:], sn_b)
                    nc.vector.tensor_add(dst, t1[:], t2[:])

                def krope_vt(b):
                    """Rope K into kT_r and PE-transpose V into v_r for batch b."""
                    for j in range(QB):
                        off = b * S + j * 512
                        rope(kT_r[:, off:off + 512][:, None, :],
                             qkv_sb[:, NQ:NQ + 1, off:off + 512], j * 512, 1)
                        for jj in range(512 // P):
                            so = off // P + jj
                            tps = ps_tile([P, P], BF, name="vt", tag="vt", bufs=2)
                            nc.tensor.transpose(
                                tps[:], qkv_sb[:, NQ + 1, off + jj * P:off + (jj + 1) * P],
                                ident_b[:])
                            nc.vector.tensor_copy(v_r[:, so, :], tps[:])

                # ---------------- Phase 1: qkv^T = w3^T @ x^T ----------------
                NHB = SQ // 256
                ps1_cm = tc.tile_pool(name="ps1", bufs=1, space="PSUM")
                pspool["cur"] = ps1_cm.__enter__()
                with tc.tile_pool(name="p1x", bufs=2) as p1x:
                    for hb in range(NHB):
                        xt = p1x.tile([P, HCH, 256], BF, name="xt", tag="xt")
                        nc.sync.dma_start(xt[:], xT_v[:, :, hb * 256:(hb + 1) * 256])
                        for ci in range(C6):
                            p1 = ps_tile([P, 256], F32, name="p1p", tag="p1p",
                                         bufs=6)
                            for hc in range(HCH):
                                nc.tensor.matmul(
                                    p1[:], w3t[:, hc, ci * P:(ci + 1) * P],
                                    xt[:, hc, :],
                                    start=(hc == 0), stop=(hc == HCH - 1))
                            nc.vector.tensor_copy(
                                qkv_sb[:, ci, hb * 256:(hb + 1) * 256], p1[:])
                        if hb == NHB // 2 - 1:
                            krope_vt(0)
                        if hb == NHB - 1:
                            krope_vt(1)

                # ---------------- Phase 2+3: attention, out-proj quarters ----
                ps1_cm.__exit__(None, None, None)
                ps2_cm = tc.tile_pool(name="ps2", bufs=1, space="PSUM")
                pspool["cur"] = ps2_cm.__enter__()
                ap_cm = tc.tile_pool(name="ap", bufs=1)
                ap = ap_cm.__enter__()

                def attention_block(b, qb, qr):
                    nch = (qb + 1) * 4
                    qoff = b * S + qb * 512
                    for hp in range(NQ // 2):
                        h0 = 2 * hp
                        pt = ap.tile([P, 2, 4, 512], BF, name="pT", tag="pT",
                                     bufs=2)
                        lps = ps_tile([P, 2, 512], F32, name="lp", tag="lp")
                        ops = ps_tile([P, 2, 512], F32, name="av", tag="av")

                        def lps_av(kc):
                            for j in range(2):
                                nc.tensor.matmul(
                                    lps[:, j, :], ones_b[:], pt[:, j, kc % 4, :],
                                    start=(kc == 0), stop=(kc == nch - 1))
                                nc.tensor.matmul(
                                    ops[:, j, :], v_r[:, b * SCH + kc, :],
                                    pt[:, j, kc % 4, :],
                                    start=(kc == 0), stop=(kc == nch - 1))

                        for kc in range(nch):
                            diag = kc >= nch - 4
                            sc = ps_tile([P, 2, 512], F32, name="sc", tag="sc",
                                         bufs=2)
                            for j in range(2):
                                nc.tensor.matmul(
                                    sc[:, j, :],
                                    kT_r[:, b * S + kc * P:b * S + (kc + 1) * P],
                                    qr[:, h0 + j, :], start=True, stop=not diag)
                                if diag:
                                    nc.tensor.matmul(
                                        sc[:, j, :], ident_b[:],
                                        bias_t[:, kc - (nch - 4), :],
                                        start=False, stop=True)
                            nc.scalar.activation(
                                pt[:, :, kc % 4, :], sc[:],
                                mybir.ActivationFunctionType.Exp)
                            if kc >= 2:
                                lps_av(kc - 2)
                        lps_av(max(nch - 2, 0))
                        if nch > 1:
                            lps_av(nch - 1)
                        # reciprocal on ACT: 1/x = exp(-ln(x)); then divide+store
                        lnt = ap.tile([P, 2, 512], F32, name="lnt", tag="lnt")
                        nc.scalar.activation(
                            lnt[:], lps[:], mybir.ActivationFunctionType.Ln)
                        rec = ap.tile([P, 2, 512], BF, name="rec", tag="rec")
                        nc.scalar.activation(
                            rec[:], lnt[:], mybir.ActivationFunctionType.Exp,
                            scale=-1.0)
                        att_o = ap.tile([P, 2, 512], BF, name="atto", tag="atto",
                                        bufs=2)
                        nc.vector.tensor_mul(att_o[:], ops[:], rec[:])
                        nc.sync.dma_start(
                            attnT_sp[:, h0:h0 + 2, qoff:qoff + 512], att_o[:])

                def phase3_quarter(q4):
                    att_all = ap.tile([P, NQ, 1024], BF, name="attall",
                                      tag="attall")
                    nc.sync.dma_start(
                        att_all[:], attnT_sp[:, :, q4 * 1024:(q4 + 1) * 1024])
                    for htg in range(HCH // 4):
                        wg = ap.tile([P, NQ, 512], BF, name="wg", tag="wg",
                                     bufs=2)
                        nc.sync.dma_start(
                            wg[:], wout_v[:, :, htg * 512:(htg + 1) * 512])
                        for hl in range(4):
                            ht = htg * 4 + hl
                            o3 = ps_tile([P, 2, 512], F32, name="o3", tag="sc",
                                         bufs=2)
                            for ci in range(NQ):
                                for sb in range(2):
                                    nc.tensor.matmul(
                                        o3[:, sb, :],
                                        wg[:, ci, hl * P:(hl + 1) * P],
                                        att_all[:, ci, sb * 512:(sb + 1) * 512],
                                        start=(ci == 0), stop=(ci == NQ - 1))
                            ost = ap.tile([P, 2, 512], F16, name="ost", tag="ost",
                                          bufs=2)
                            nc.vector.tensor_copy(ost[:], o3[:])
                            nc.sync.dma_start(
                                outT_v[:, ht, q4 * 2:q4 * 2 + 2, :], ost[:])

                blocks = [(b, qb) for b in range(B) for qb in range(QB)]
                qr_tiles = {}

                def qrope(i):
                    b, qb = blocks[i]
                    qr = rp.tile([P, NQ, 512], BF, name="qr", tag="qr", bufs=2)
                    rope(qr[:, :, :], qkv_sb[:, 0:NQ, b * S + qb * 512:b * S + (qb + 1) * 512],
                         qb * 512, NQ)
                    qr_tiles[i] = qr

                qrope(0)
                for i, (b, qb) in enumerate(blocks):
                    if i + 1 < len(blocks):
                        qrope(i + 1)
                    attention_block(b, qb, qr_tiles.pop(i))
                    if b == 1 and qb > 0:
                        phase3_quarter(qb - 1)
                phase3_quarter(3)
                ap_cm.__exit__(None, None, None)
                ps2_cm.__exit__(None, None, None)

            if reps == 1:
                body()
            else:
                with tc.For_i(0, reps, 1) as iv:
                    body(iv)
    return nc


def host_inputs(x, cos, sin, Wqkv, Wout, cfg=CFG):
    """Build the 8 per-core input maps from the full-problem inputs."""
    import ml_dtypes
    BF_NP = ml_dtypes.bfloat16
    B, S, H, HD, NQ = cfg["B"], cfg["S"], cfg["H"], cfg["HD"], cfg["NQ"]
    SQ = B * S
    NH = NQ * N_CORES          # total q heads
    scale = 1.0 / np.sqrt(HD)

    x = np.asarray(x, dtype=np.float32)
    cos = np.asarray(cos, dtype=np.float32)
    sin = np.asarray(sin, dtype=np.float32)
    Wqkv = np.asarray(Wqkv, dtype=np.float32)
    Wout = np.asarray(Wout, dtype=np.float32)

    xT_b = np.ascontiguousarray(x.reshape(SQ, H).T).astype(BF_NP)
    cosT = np.ascontiguousarray(cos.T).astype(BF_NP)
    sinT = sin.T
    sinS = np.ascontiguousarray(
        np.concatenate([-sinT[:HD // 2], sinT[HD // 2:]], axis=0)).astype(BF_NP)
    qv = np.arange(512)
    pv = np.arange(P)
    bias = np.zeros((P, 4, 512), np.float32)
    for v in range(4):
        bias[:, v, :] = np.where(qv[None, :] >= (v * P + pv)[:, None], 0.0, -60.0)
    bias = bias.reshape(P, 4 * 512).astype(BF_NP)

    in_maps = []
    for c in range(N_CORES):
        wq = Wqkv[:, c * NQ * HD:(c + 1) * NQ * HD] * scale
        wk = Wqkv[:, NH * HD + c * HD: NH * HD + (c + 1) * HD]
        wv = Wqkv[:, NH * HD + N_CORES * HD + c * HD: NH * HD + N_CORES * HD + (c + 1) * HD]
        w3 = np.concatenate([wq, wk, wv], axis=1).astype(BF_NP)
        wout = Wout[c * NQ * HD:(c + 1) * NQ * HD, :].astype(BF_NP)
        in_maps.append({
            "xT": xT_b, "w3": w3, "wout": wout,
            "cosT": cosT, "sinS": sinS, "biasT": bias,
        })
    return in_maps


class _Runner:
    """Compiled-kernel runner over the axon PJRT path (kept for re-invocation)."""

    def __init__(self, nc, n_cores):
        import jax
        from jax.sharding import Mesh, PartitionSpec
        from jax.experimental.shard_map import shard_map
        from concourse.bass2jax import (
            _bass_exec_p, partition_id_tensor, install_neuronx_cc_hook)
        install_neuronx_cc_hook()
        self.jax = jax
        self.n_cores = n_cores
        partition_name = nc.partition_id_tensor.name if nc.partition_id_tensor else None
        in_names, out_names, out_avals, zero_outs = [], [], [], []
        for alloc in nc.m.functions[0].allocations:
            if not isinstance(alloc, mybir.MemoryLocationSet):
                continue
            name = alloc.memorylocations[0].name
            if alloc.kind == "ExternalInput":
                if name != partition_name:
                    in_names.append(name)
            elif alloc.kind == "ExternalOutput":
                shape = tuple(alloc.tensor_shape)
                dtype = mybir.dt.np(alloc.dtype)
                out_avals.append(jax.core.ShapedArray(shape, dtype))
                out_names.append(name)
                zero_outs.append(np.zeros(shape, dtype))
        self.in_names = in_names[:]
        self.out_names, self.out_avals, self.zero_outs = out_names, out_avals, zero_outs
        self.n_params = len(in_names)
        all_names = in_names + out_names
        if partition_name is not None:
            all_names.append(partition_name)

        def _body(*args):
            operands = list(args)
            if partition_name is not None:
                operands.append(partition_id_tensor())
            outs = _bass_exec_p.bind(
                *operands, out_avals=tuple(out_avals), in_names=tuple(all_names),
                out_names=tuple(out_names), lowering_input_output_aliases=(),
                sim_require_finite=True, sim_require_nnan=True, nc=nc)
            return tuple(outs)

        devices = jax.devices()[:n_cores]
        self.mesh = Mesh(np.asarray(devices), ("core",))
        specs_in = (PartitionSpec("core"),) * (self.n_params + len(out_names))
        specs_out = (PartitionSpec("core"),) * len(out_names)
        self.sharded = jax.jit(
            shard_map(_body, mesh=self.mesh, in_specs=specs_in,
                      out_specs=specs_out, check_rep=False),
            keep_unused=True)
        self._dev_args = None

    def stage(self, in_maps):
        import jax
        from jax.sharding import PartitionSpec
        per_core = [[np.asarray(m[n]) for n in self.in_names] for m in in_maps]
        concat = [np.concatenate([per_core[c][i] for c in range(self.n_cores)], axis=0)
                  for i in range(self.n_params)]
        concat += [np.zeros((self.n_cores * z.shape[0], *z.shape[1:]), z.dtype)
                   for z in self.zero_outs]
        sh = jax.sharding.NamedSharding(self.mesh, PartitionSpec("core"))
        self._dev_args = [jax.device_put(a, sh) for a in concat]
        jax.block_until_ready(self._dev_args)

    def execute(self):
        out = self.sharded(*self._dev_args)
        self.jax.block_until_ready(out)
        return out

    def results(self, out):
        return [
            {n: np.asarray(out[i]).reshape(self.n_cores, *self.out_avals[i].shape)[c]
             for i, n in enumerate(self.out_names)}
            for c in range(self.n_cores)
        ]


_cached = {}


def _get_runner(reps=1):
    key = reps
    if key not in _cached:
        nc = build(CFG, reps=reps)
        nc.compile()
        _cached[key] = _Runner(nc, N_CORES)
    return _cached[key]


def kernel(x, cos, sin, Wqkv, Wout):
    cfg = CFG
    B, S, H = cfg["B"], cfg["S"], cfg["H"]
    runner = _get_runner(reps=1)
    in_maps = host_inputs(x, cos, sin, Wqkv, Wout, cfg)
    runner.stage(in_maps)
    out = runner.execute()
    results = runner.results(out)
    acc = np.zeros((B * S, H), np.float32)
    for c in range(N_CORES):
        acc += results[c]["outT"].T.astype(np.float32)
    return acc.reshape(B, S, H).astype(np.float32)


# revision 21
# speedup vs baseline: 1.6124x; 1.5566x over previous
"""Causal self-attention (GQA, RoPE) on 8 Trainium2 NeuronCores.

Sharding: tensor-parallel by KV-head group. Core c owns kv-head c and its 4
query heads, for both batch elements. Each core computes:
  qkv^T slice -> RoPE -> causal attention -> out-projection partial
The host sums the 8 partial out-projection results (Wout row-sharded), which
replaces the all-reduce.

All matmuls run in bf16 (PE peak rate, fast weight load); PSUM accumulation is
fp32. Causal masking is an additive -60 bias folded into the scores PSUM via an
identity-weight matmul (exp(-60)~1e-26, vanishes in bf16). The softmax
reciprocal runs on the Scalar engine as exp(-ln(x)) (same ACT table set as the
exp). qkv stays resident in SBUF (no DRAM round-trip); attention output makes
one bf16 round-trip so the out-projection can rerun with 4-block PSUM
accumulators (weight loads amortized over 4 matmuls).

Layouts (per core, s = b*S + pos, SQ = B*S):
  xT    [H, SQ]   bf16  x transposed
  w3    [H, 768]  bf16  [Wq(4 heads, pre-scaled by 1/sqrt(hd)) | Wk | Wv]
  wout  [512, H]  bf16  Wout rows for this core's 4 q heads
  cosT  [128, S]  bf16  cos table transposed (per-position, shared by batches)
  sinS  [128, S]  bf16  sin table, rows 0:64 negated (rotate_half baked in)
  biasT [128, 4*512] bf16  additive causal bias (0 valid / -60 masked) for the
                           4 diagonal k-chunks of a 512-wide q block
Scratch (DRAM): attnT_sp [128, 4, SQ] bf16.
Output: outT [H, SQ] fp16 (partial out-projection, transposed; host sums fp32).
"""
import numpy as np

import concourse.bass as bass
import concourse.mybir as mybir
import concourse.tile as tile
from concourse import bacc
from concourse.masks import make_identity

F32 = mybir.dt.float32
BF = mybir.dt.bfloat16
F16 = mybir.dt.float16
P = 128

N_CORES = 8
CFG = dict(B=2, S=2048, H=4096, HD=128, NQ=4)  # NQ = q heads per core


def build(cfg=CFG, reps=1):
    B, S, H, HD, NQ = cfg["B"], cfg["S"], cfg["H"], cfg["HD"], cfg["NQ"]
    SQ = B * S
    HCH = H // P          # h chunks (contraction tiles)
    C6 = NQ + 2           # c-tiles: NQ q heads, 1 k, 1 v
    CW = C6 * P           # qkv out width per core
    NSB = SQ // 512       # 512-wide s blocks
    QB = S // 512         # q blocks per batch
    SCH = S // P          # k chunks per batch
    h2 = HD // 2

    nc = bacc.Bacc("TRN2", target_bir_lowering=False, debug=False,
                   num_devices=N_CORES)
    xT = nc.dram_tensor("xT", [H, SQ], BF, kind="ExternalInput").ap()
    w3 = nc.dram_tensor("w3", [H, CW], BF, kind="ExternalInput").ap()
    wout = nc.dram_tensor("wout", [NQ * P, H], BF, kind="ExternalInput").ap()
    cosT = nc.dram_tensor("cosT", [P, S], BF, kind="ExternalInput").ap()
    sinS = nc.dram_tensor("sinS", [P, S], BF, kind="ExternalInput").ap()
    biasT = nc.dram_tensor("biasT", [P, 4 * 512], BF, kind="ExternalInput").ap()
    outT = nc.dram_tensor("outT", [H, SQ], F16, kind="ExternalOutput").ap()

    xT_v = xT.rearrange("(ho p) s -> p ho s", p=P)        # [128, HCH, SQ]
    w3_v = w3.rearrange("(ho p) c -> p ho c", p=P)        # [128, HCH, CW]
    wout_v = wout.rearrange("(co p) n -> p co n", p=P)    # [128, NQ, H]
    outT_v = outT.rearrange("(ho p) (hf q) -> p ho hf q", p=P, q=512)

    with tile.TileContext(nc, pool_alloc_mode="queue") as tc:
        with tc.tile_pool(name="dram", bufs=1, space="DRAM") as dram, \
             tc.tile_pool(name="const", bufs=1) as cp:
            attnT_sp = [dram.tile([P, NQ, 1024], BF, name=f"att_sp{q}")
                        for q in range(4)]

            # ---- persistent constants/weights (outside the reps loop) ----
            w3t = cp.tile([P, HCH, CW], BF)
            nc.sync.dma_start(w3t[:], w3_v)
            cos_t = cp.tile([P, S], BF)
            nc.sync.dma_start(cos_t[:], cosT)
            sin_t = cp.tile([P, S], BF)
            nc.sync.dma_start(sin_t[:], sinS)
            bias_t = cp.tile([P, 4, 512], BF)
            nc.sync.dma_start(bias_t[:], biasT.rearrange("p (v q) -> p v q", v=4))
            ident_b = cp.tile([P, P], BF)
            ones_b = cp.tile([P, P], BF)
            with tc.tile_pool(name="init", bufs=1) as ip:
                idf = ip.tile([P, P], F32)
                make_identity(nc, idf[:])
                nc.vector.tensor_copy(ident_b[:], idf[:])
                onef = ip.tile([P, P], F32)
                nc.vector.memset(onef[:], 1.0)
                nc.vector.tensor_copy(ones_b[:], onef[:])

            def body(iv=None):
              pspool = {}

              def ps_tile(*a, **kw):
                  return pspool["cur"].tile(*a, **kw)

              with tc.tile_pool(name="span", bufs=1) as sp, \
                   tc.tile_pool(name="rp", bufs=1) as rp:
                # per-512-block q/k/v tiles + per-batch roped-K / V^T tiles:
                # fine granularity lets the For_i loop overlap rep i+1's
                # phase 1 with rep i's attention tail (whole-tile WAR
                # otherwise serializes the loop).
                q_sb = [sp.tile([P, NQ, 512], BF, name=f"qsb{j}")
                        for j in range(NSB)]
                k_sb = [sp.tile([P, 512], BF, name=f"ksb{j}") for j in range(NSB)]
                v_sb = [sp.tile([P, 512], BF, name=f"vsb{j}") for j in range(NSB)]
                kT_b = [sp.tile([P, S], BF, name=f"ktb{b}") for b in range(B)]
                v_rb = [sp.tile([P, SCH, HD], BF, name=f"vrb{b}") for b in range(B)]

                def rope(dst, src_ap, coff, n):
                    """dst[:, :n, 512] = rope(src) using cos/sin cols coff."""
                    qrt = rp.tile([P, n, 512], BF, name="qrt", tag=f"qrt{n}")
                    nc.vector.tensor_copy(qrt[:h2, :, :], src_ap[h2:2 * h2, :n, :])
                    nc.vector.tensor_copy(qrt[h2:2 * h2, :, :], src_ap[:h2, :n, :])
                    cs_b = cos_t[:, coff:coff + 512][:, None, :].to_broadcast((P, n, 512))
                    sn_b = sin_t[:, coff:coff + 512][:, None, :].to_broadcast((P, n, 512))
                    t1 = rp.tile([P, n, 512], BF, name="rt1", tag=f"rt1{n}")
                    t2 = rp.tile([P, n, 512], BF, name="rt2", tag=f"rt2{n}")
                    nc.vector.tensor_mul(t1[:], src_ap[:, :n, :], cs_b)
                    nc.vector.tensor_mul(t2[:], qrt[:], sn_b)
                    nc.vector.tensor_add(dst, t1[:], t2[:])

                def krope_vt(b):
                    """Rope K into kT_b and PE-transpose V into v_rb for batch b."""
                    for j in range(QB):
                        jb = b * QB + j
                        off = j * 512
                        rope(kT_b[b][:, off:off + 512][:, None, :],
                             k_sb[jb][:, None, :], j * 512, 1)
                        for jj in range(512 // P):
                            so = off // P + jj
                            tps = ps_tile([P, P], BF, name="vt", tag="vt", bufs=2)
                            nc.tensor.transpose(
                                tps[:], v_sb[jb][:, jj * P:(jj + 1) * P],
                                ident_b[:])
                            nc.vector.tensor_copy(v_rb[b][:, so, :], tps[:])

                # ---------------- Phase 1: qkv^T = w3^T @ x^T ----------------
                NHB = SQ // 256
                ps1_cm = tc.tile_pool(name="ps1", bufs=1, space="PSUM")
                pspool["cur"] = ps1_cm.__enter__()
                with tc.tile_pool(name="p1x", bufs=2) as p1x:
                    for hb in range(NHB):
                        xt = p1x.tile([P, HCH, 256], BF, name="xt", tag="xt", bufs=3)
                        nc.sync.dma_start(xt[:], xT_v[:, :, hb * 256:(hb + 1) * 256])
                        for ci in range(C6):
                            p1 = ps_tile([P, 256], F32, name="p1p", tag="p1p",
                                         bufs=3)
                            for hc in range(HCH):
                                nc.tensor.matmul(
                                    p1[:], w3t[:, hc, ci * P:(ci + 1) * P],
                                    xt[:, hc, :],
                                    start=(hc == 0), stop=(hc == HCH - 1))
                            j, half = hb // 2, (hb % 2) * 256
                            if ci < NQ:
                                dst = q_sb[j][:, ci, half:half + 256]
                            elif ci == NQ:
                                dst = k_sb[j][:, half:half + 256]
                            else:
                                dst = v_sb[j][:, half:half + 256]
                            nc.vector.tensor_copy(dst, p1[:])
                        if hb == NHB // 2 - 1:
                            krope_vt(0)
                        if hb == NHB - 1:
                            krope_vt(1)

                # ---------------- Phase 2+3: attention, out-proj quarters ----
                ps1_cm.__exit__(None, None, None)
                ps2_cm = tc.tile_pool(name="ps2", bufs=1, space="PSUM")
                pspool["cur"] = ps2_cm.__enter__()
                ap_cm = tc.tile_pool(name="ap", bufs=1)
                ap = ap_cm.__enter__()

                def attention_block(b, qb, qr):
                    nch = (qb + 1) * 4
                    qoff = b * S + qb * 512
                    for hp in range(NQ // 2):
                        h0 = 2 * hp
                        pt = ap.tile([P, 2, 3, 512], BF, name="pT", tag="pT",
                                     bufs=2)
                        trims = [0, 0, 0]
                        lps = ps_tile([P, 2, 512], F32, name="lp", tag="lp")
                        ops = ps_tile([P, 2, 512], F32, name="av", tag="av")

                        def lps_av(kc):
                            vr = trims[kc % 3]
                            for j in range(2):
                                nc.tensor.matmul(
                                    lps[:, j, vr:], ones_b[:],
                                    pt[:, j, kc % 3, vr:],
                                    start=(kc == 0), stop=(kc == nch - 1),
                                    skip_group_check=True)
                                nc.tensor.matmul(
                                    ops[:, j, vr:], v_rb[b][:, kc, :],
                                    pt[:, j, kc % 3, vr:],
                                    start=(kc == 0), stop=(kc == nch - 1),
                                    skip_group_check=True)

                        for kc in range(nch):
                            diag = kc >= nch - 4
                            # diagonal chunk kc==nch-4+v: columns [0:128v) are
                            # fully masked -> skip them in scores/exp/lps/av
                            vr = (kc - (nch - 4)) * P if diag else 0
                            trims[kc % 3] = vr
                            sc = ps_tile([P, 2, 512], F32, name="sc", tag="sc",
                                         bufs=2)
                            for j in range(2):
                                nc.tensor.matmul(
                                    sc[:, j, vr:],
                                    kT_b[b][:, kc * P:(kc + 1) * P],
                                    qr[:, h0 + j, vr:], start=True,
                                    stop=not diag)
                                if vr:
                                    nc.tensor.matmul(
                                        sc[:, j, vr:], ident_b[:],
                                        bias_t[:, kc - (nch - 4), vr:],
                                        start=False, stop=True)
                                elif diag:
                                    nc.tensor.matmul(
                                        sc[:, j, :], ident_b[:],
                                        bias_t[:, 0, :],
                                        start=False, stop=True)
                            nc.scalar.activation(
                                pt[:, :, kc % 3, vr:], sc[:, :, vr:],
                                mybir.ActivationFunctionType.Exp)
                            if kc >= 2:
                                lps_av(kc - 2)
                        lps_av(max(nch - 2, 0))
                        if nch > 1:
                            lps_av(nch - 1)
                        # quick PSUM->SBUF copies free lps/ops banks for the
                        # next pair; reciprocal + divide then run off-SBUF
                        lps_s = ap.tile([P, 2, 512], F32, name="lpss", tag="lpss")
                        nc.vector.tensor_copy(lps_s[:], lps[:])
                        ops_s = ap.tile([P, 2, 512], F32, name="opss", tag="opss")
                        nc.vector.tensor_copy(ops_s[:], ops[:])
                        rec = ap.tile([P, 2, 512], F32, name="rec", tag="rec")
                        nc.vector.reciprocal_approx_fast(rec[:], lps_s[:])
                        att_o = ap.tile([P, 2, 512], BF, name="atto", tag="atto")
                        nc.vector.tensor_mul(att_o[:], ops_s[:], rec[:])
                        nc.scalar.dma_start(
                            attnT_sp[qoff // 1024][:, h0:h0 + 2,
                                                   qoff % 1024:qoff % 1024 + 512],
                            att_o[:])

                def phase3_quarter(q4):
                    att_all = ap.tile([P, NQ, 1024], BF, name="attall",
                                      tag="attall", bufs=2)
                    nc.sync.dma_start(att_all[:], attnT_sp[q4][:])
                    for htg in range(HCH // 4):
                        wg = ap.tile([P, NQ, 512], BF, name="wg", tag="wg",
                                     bufs=2)
                        nc.sync.dma_start(
                            wg[:], wout_v[:, :, htg * 512:(htg + 1) * 512])
                        for hl in range(4):
                            ht = htg * 4 + hl
                            o3 = ps_tile([P, 2, 512], F32, name="o3", tag="sc",
                                         bufs=2)
                            for ci in range(NQ):
                                for sb in range(2):
                                    nc.tensor.matmul(
                                        o3[:, sb, :],
                                        wg[:, ci, hl * P:(hl + 1) * P],
                                        att_all[:, ci, sb * 512:(sb + 1) * 512],
                                        start=(ci == 0), stop=(ci == NQ - 1))
                            ost = ap.tile([P, 2, 512], F16, name="ost", tag="ost",
                                          bufs=2)
                            nc.vector.tensor_copy(ost[:], o3[:])
                            nc.scalar.dma_start(
                                outT_v[:, ht, q4 * 2:q4 * 2 + 2, :], ost[:])

                blocks = [(b, qb) for b in range(B) for qb in range(QB)]
                qr_tiles = {}

                def qrope(i):
                    b, qb = blocks[i]
                    qr = rp.tile([P, NQ, 512], BF, name="qr", tag="qr", bufs=2)
                    rope(qr[:, :, :], q_sb[b * QB + qb][:, :, :], qb * 512, NQ)
                    qr_tiles[i] = qr

                qrope(0)
                for i, (b, qb) in enumerate(blocks):
                    if i + 1 < len(blocks):
                        qrope(i + 1)
                    attention_block(b, qb, qr_tiles.pop(i))
                    if b == 1 and qb > 0:
                        phase3_quarter(qb - 1)
                phase3_quarter(3)
                ap_cm.__exit__(None, None, None)
                ps2_cm.__exit__(None, None, None)

            if reps == 1:
                body()
            else:
                with tc.For_i(0, reps, 1) as iv:
                    body(iv)
    return nc


def host_inputs(x, cos, sin, Wqkv, Wout, cfg=CFG):
    """Build the 8 per-core input maps from the full-problem inputs."""
    import ml_dtypes
    BF_NP = ml_dtypes.bfloat16
    B, S, H, HD, NQ = cfg["B"], cfg["S"], cfg["H"], cfg["HD"], cfg["NQ"]
    SQ = B * S
    NH = NQ * N_CORES          # total q heads
    scale = 1.0 / np.sqrt(HD)

    x = np.asarray(x, dtype=np.float32)
    cos = np.asarray(cos, dtype=np.float32)
    sin = np.asarray(sin, dtype=np.float32)
    Wqkv = np.asarray(Wqkv, dtype=np.float32)
    Wout = np.asarray(Wout, dtype=np.float32)

    xT_b = np.ascontiguousarray(x.reshape(SQ, H).T).astype(BF_NP)
    cosT = np.ascontiguousarray(cos.T).astype(BF_NP)
    sinT = sin.T
    sinS = np.ascontiguousarray(
        np.concatenate([-sinT[:HD // 2], sinT[HD // 2:]], axis=0)).astype(BF_NP)
    qv = np.arange(512)
    pv = np.arange(P)
    bias = np.zeros((P, 4, 512), np.float32)
    for v in range(4):
        bias[:, v, :] = np.where(qv[None, :] >= (v * P + pv)[:, None], 0.0, -60.0)
    bias = bias.reshape(P, 4 * 512).astype(BF_NP)

    in_maps = []
    for c in range(N_CORES):
        wq = Wqkv[:, c * NQ * HD:(c + 1) * NQ * HD] * scale
        wk = Wqkv[:, NH * HD + c * HD: NH * HD + (c + 1) * HD]
        wv = Wqkv[:, NH * HD + N_CORES * HD + c * HD: NH * HD + N_CORES * HD + (c + 1) * HD]
        w3 = np.concatenate([wq, wk, wv], axis=1).astype(BF_NP)
        wout = Wout[c * NQ * HD:(c + 1) * NQ * HD, :].astype(BF_NP)
        in_maps.append({
            "xT": xT_b, "w3": w3, "wout": wout,
            "cosT": cosT, "sinS": sinS, "biasT": bias,
        })
    return in_maps


class _Runner:
    """Compiled-kernel runner over the axon PJRT path (kept for re-invocation)."""

    def __init__(self, nc, n_cores):
        import jax
        from jax.sharding import Mesh, PartitionSpec
        from jax.experimental.shard_map import shard_map
        from concourse.bass2jax import (
            _bass_exec_p, partition_id_tensor, install_neuronx_cc_hook)
        install_neuronx_cc_hook()
        self.jax = jax
        self.n_cores = n_cores
        partition_name = nc.partition_id_tensor.name if nc.partition_id_tensor else None
        in_names, out_names, out_avals, zero_outs = [], [], [], []
        for alloc in nc.m.functions[0].allocations:
            if not isinstance(alloc, mybir.MemoryLocationSet):
                continue
            name = alloc.memorylocations[0].name
            if alloc.kind == "ExternalInput":
                if name != partition_name:
                    in_names.append(name)
            elif alloc.kind == "ExternalOutput":
                shape = tuple(alloc.tensor_shape)
                dtype = mybir.dt.np(alloc.dtype)
                out_avals.append(jax.core.ShapedArray(shape, dtype))
                out_names.append(name)
                zero_outs.append(np.zeros(shape, dtype))
        self.in_names = in_names[:]
        self.out_names, self.out_avals, self.zero_outs = out_names, out_avals, zero_outs
        self.n_params = len(in_names)
        all_names = in_names + out_names
        if partition_name is not None:
            all_names.append(partition_name)

        def _body(*args):
            operands = list(args)
            if partition_name is not None:
                operands.append(partition_id_tensor())
            outs = _bass_exec_p.bind(
                *operands, out_avals=tuple(out_avals), in_names=tuple(all_names),
                out_names=tuple(out_names), lowering_input_output_aliases=(),
                sim_require_finite=True, sim_require_nnan=True, nc=nc)
            return tuple(outs)

        devices = jax.devices()[:n_cores]
        self.mesh = Mesh(np.asarray(devices), ("core",))
        specs_in = (PartitionSpec("core"),) * (self.n_params + len(out_names))
        specs_out = (PartitionSpec("core"),) * len(out_names)
        self.sharded = jax.jit(
            shard_map(_body, mesh=self.mesh, in_specs=specs_in,
                      out_specs=specs_out, check_rep=False),
            keep_unused=True)
        self._dev_args = None

    def stage(self, in_maps):
        import jax
        from jax.sharding import PartitionSpec
        per_core = [[np.asarray(m[n]) for n in self.in_names] for m in in_maps]
        concat = [np.concatenate([per_core[c][i] for c in range(self.n_cores)], axis=0)
                  for i in range(self.n_params)]
        concat += [np.zeros((self.n_cores * z.shape[0], *z.shape[1:]), z.dtype)
                   for z in self.zero_outs]
        sh = jax.sharding.NamedSharding(self.mesh, PartitionSpec("core"))
        self._dev_args = [jax.device_put(a, sh) for a in concat]
        jax.block_until_ready(self._dev_args)

    def execute(self):
        out = self.sharded(*self._dev_args)
        self.jax.block_until_ready(out)
        return out

    def results(self, out):
        return [
            {n: np.asarray(out[i]).reshape(self.n_cores, *self.out_avals[i].shape)[c]
             for i, n in enumerate(self.out_names)}
            for c in range(self.n_cores)
        ]


_cached = {}


def _get_runner(reps=1):
    key = reps
    if key not in _cached:
        nc = build(CFG, reps=reps)
        nc.compile()
        _cached[key] = _Runner(nc, N_CORES)
    return _cached[key]


def kernel(x, cos, sin, Wqkv, Wout):
    cfg = CFG
    B, S, H = cfg["B"], cfg["S"], cfg["H"]
    runner = _get_runner(reps=1)
    in_maps = host_inputs(x, cos, sin, Wqkv, Wout, cfg)
    runner.stage(in_maps)
    out = runner.execute()
    results = runner.results(out)
    acc = np.zeros((B * S, H), np.float32)
    for c in range(N_CORES):
        acc += results[c]["outT"].T.astype(np.float32)
    return acc.reshape(B, S, H).astype(np.float32)
